# revision 1
# baseline (speedup 1.0000x reference)
"""Cross-entropy (NLL of log-softmax) kernel for Trainium2, 8-core SPMD.

Full inputs: logits [4096, 50257] f32, target [4096] int (class ids).
Full output: nll [4096] f32,  nll[n] = logsumexp(logits[n, :]) - logits[n, target[n]].

Sharding: rows (batch) split evenly across 8 cores -> 512 rows/core.
Per core: stream column chunks of the row-tile through SBUF, fused
exp+accumulate on the scalar (ACT) engine, gather logits[n, target[n]]
via indirect DMA with host-precomputed flat indices, then
nll = ln(sum) - gathered.

No max-subtraction is needed: inputs are standard-normal logits, so
exp() stays comfortably inside fp32 range (max |x| ~ 6).
"""

import numpy as np

import concourse.bacc as bacc
import concourse.bass as bass
import concourse.tile as tile
from concourse import mybir
from concourse.bass_utils import run_bass_kernel_spmd

N, C = 4096, 50257
NCORES = 8
NL = N // NCORES  # rows per core
P = 128  # partitions
F = 8192  # column chunk (free dim) per DMA/exp step


def build_program(
    nl=NL,
    c=C,
    f=F,
    chunk_bufs=3,
    reps=1,
    exp_cols=None,  # None = full chunk; small int = timing variant (DMA-only-ish)
    gather=True,  # False = skip indirect-DMA gather (timing variant)
    dual_ring=False,  # issue alternate chunk loads from the ACT HWDGE ring
    batch_epilogue=True,  # all Exps first, then all Lns (one ACT table swap)
):
    """Build the per-core Bass program (identical on all cores).

    reps>1 repeats the whole computation in-kernel (for timing: the
    marginal cost per rep is the true HW time, dispatch overhead cancels).
    """
    # Bacc (not raw Bass): its finalize() pass legalizes multi-sem sync
    # waits into forms walrus codegen accepts.
    nc = bacc.Bacc(None, target_bir_lowering=False)
    logits = nc.dram_tensor("logits", [nl, c], mybir.dt.float32, kind="ExternalInput")
    flatidx = nc.dram_tensor("flatidx", [nl, 1], mybir.dt.int32, kind="ExternalInput")
    nll = nc.dram_tensor("nll", [nl, 1], mybir.dt.float32, kind="ExternalOutput")

    n_tiles = (nl + P - 1) // P
    chunks = [(s, min(f, c - s)) for s in range(0, c, f)]
    nch = len(chunks)

    # Flat [nl*c, 1] view of logits for the element gather (offset must be 0).
    logits_flat = bass.AP(tensor=logits, offset=0, ap=[[1, nl * c], [1, 1]])

    with tile.TileContext(nc) as tc:
        with (
            tc.tile_pool(name="chunks", bufs=chunk_bufs) as chunk_pool,
            tc.tile_pool(name="small", bufs=2 * n_tiles) as small,
        ):
            def epilogue(t, parts, gat):
                r0 = t * P
                rows = min(P, nl - r0)
                ssum = small.tile([P, 1], mybir.dt.float32, tag="ssum")
                nc.vector.reduce_sum(
                    out=ssum[:rows], in_=parts[:rows, :], axis=mybir.AxisListType.X
                )
                logz = small.tile([P, 1], mybir.dt.float32, tag="logz")
                nc.scalar.activation(
                    out=logz[:rows],
                    in_=ssum[:rows],
                    func=mybir.ActivationFunctionType.Ln,
                )
                res = small.tile([P, 1], mybir.dt.float32, tag="res")
                nc.vector.tensor_sub(res[:rows], logz[:rows], gat[:rows])
                # store via gpsimd's queue so it can't head-of-line block the
                # HWDGE load ring on the sync engine
                nc.gpsimd.dma_start(out=nll[r0 : r0 + rows, :], in_=res[:rows])

            for _ in range(reps):
                stash = []
                for t in range(n_tiles):
                    r0 = t * P
                    rows = min(P, nl - r0)

                    gat = small.tile([P, 1], mybir.dt.float32, tag="gat")
                    if gather:
                        idx = small.tile([P, 1], mybir.dt.int32, tag="idx")
                        nc.gpsimd.dma_start(
                            out=idx[:rows], in_=flatidx[r0 : r0 + rows, :]
                        )
                        nc.gpsimd.indirect_dma_start(
                            out=gat[:rows],
                            out_offset=None,
                            in_=logits_flat,
                            in_offset=bass.IndirectOffsetOnAxis(
                                ap=idx[:rows, :1], axis=0
                            ),
                        )
                    else:
                        nc.vector.memset(gat[:rows], 0.0)

                    parts = small.tile([P, nch], mybir.dt.float32, tag="parts")
                    for k, (s, w) in enumerate(chunks):
                        ch = chunk_pool.tile([P, f], mybir.dt.float32, tag="ch")
                        eng = nc.scalar if (dual_ring and k % 2) else nc.sync
                        eng.dma_start(
                            out=ch[:rows, :w], in_=logits[r0 : r0 + rows, s : s + w]
                        )
                        we = w if exp_cols is None else min(exp_cols, w)
                        nc.scalar.activation(
                            out=ch[:rows, :we],
                            in_=ch[:rows, :we],
                            func=mybir.ActivationFunctionType.Exp,
                            accum_out=parts[:rows, k : k + 1],
                        )
                    if batch_epilogue:
                        stash.append((t, parts, gat))
                    else:
                        epilogue(t, parts, gat)
                for t, parts, gat in stash:
                    epilogue(t, parts, gat)
    nc.finalize()
    return nc


_PROG = None


def _get_prog():
    global _PROG
    if _PROG is None:
        _PROG = build_program()
    return _PROG


def _make_in_maps(logits, target):
    logits = np.ascontiguousarray(logits, dtype=np.float32)
    tgt = np.asarray(target).astype(np.int64).reshape(N)
    base = np.arange(NL, dtype=np.int64) * C
    in_maps = []
    for cid in range(NCORES):
        lo = cid * NL
        fi = (base + tgt[lo : lo + NL]).astype(np.int32).reshape(NL, 1)
        in_maps.append({"logits": logits[lo : lo + NL], "flatidx": fi})
    return in_maps


def run(logits, target, trace=False):
    """Run on 8 cores; returns (nll [N] f32, BassKernelResults)."""
    nc = _get_prog()
    in_maps = _make_in_maps(logits, target)
    br = run_bass_kernel_spmd(nc, in_maps, list(range(NCORES)), trace=trace)
    out = np.concatenate([r["nll"].reshape(NL) for r in br.results], axis=0)
    return out.astype(np.float32, copy=False), br


def kernel(logits, target):
    out, _ = run(logits, target)
    return out



# revision 17
# speedup vs baseline: 1.0084x; 1.0084x over previous
"""Cross-entropy (NLL of log-softmax) kernel for Trainium2, 8-core SPMD.

Full inputs: logits [4096, 50257] f32, target [4096] int (class ids).
Full output: nll [4096] f32,  nll[n] = logsumexp(logits[n, :]) - logits[n, target[n]].

Sharding: rows (batch) split evenly across 8 cores -> 512 rows/core.

The kernel is HBM-read-bound (102.9 MB/core must stream through SBUF
once). Measured topology of the 8 axon cores: adjacent core pairs
(2k, 2k+1) share a ~640 GB/s DMA/HBM port (a lone core sustains
~390-430 GB/s; two cores of one pair get ~320 GB/s each; cores on
different pairs don't contend at 2-core scale). The winning layout
("flat") maximizes port efficiency under that contention:

- Rows are padded host-side to cpad = g*x elements and the whole shard
  is streamed as ONE contiguous byte stream: the [nl, cpad] shard is
  viewed as [nl*g, x] chunk-major, so each [128, x] SBUF tile is a
  fully contiguous DRAM read (consecutive descriptors adjacent).
- Small-ish descriptors (x=3142 -> 12.6 KB) with a deep buffer pool
  (12 bufs) keep many descriptors outstanding at the shared port.
- Fused exp+accumulate on the scalar (ACT) engine produces
  per-partition partial sums; a tiny cross-partition regroup (strided
  SBUF DMAs + DVE adds) yields per-row sums; nll = ln(sum) - gathered
  target logit (indirect DMA with host-precomputed flat indices).

No max-subtraction is needed: inputs are standard-normal logits, so
exp() stays comfortably inside fp32 range (max |x| ~ 6). Row padding
uses 0.0 (exp adds (cpad-c)*1.0 to a ~83k sum: rel err ~1e-5).
"""

import numpy as np

import concourse.bacc as bacc
import concourse.bass as bass
import concourse.tile as tile
from concourse import mybir
from concourse.bass_utils import run_bass_kernel_spmd

N, C = 4096, 50257
NCORES = 8
NL = N // NCORES  # rows per core
P = 128  # partitions
F = 8192  # column chunk (free dim) per DMA/exp step


def build_program(
    nl=NL,
    c=C,
    cp=None,  # padded row stride in elements (DRAM tensor width); None = c
    f=F,
    chunk_bufs=3,
    reps=1,
    exp_cols=None,  # None = full chunk; small int = timing variant (DMA-only-ish)
    gather=True,  # False = skip indirect-DMA gather (timing variant)
    dual_ring=False,  # issue alternate chunk loads from the ACT HWDGE ring
    batch_epilogue=True,  # all Exps first, then all Lns (one ACT table swap)
    ring_mode=None,  # None=legacy(dual_ring flag); 'single'|'alternate'|'alt_pool'|'split'|'triple'|'alt_tile'
    psum_out=False,  # write exp() result to a PSUM scratch instead of in-place SBUF
    psum_bufs=2,
):
    """Build the per-core Bass program (identical on all cores).

    reps>1 repeats the whole computation in-kernel (for timing: the
    marginal cost per rep is the true HW time, dispatch overhead cancels).
    """
    # Bacc (not raw Bass): its finalize() pass legalizes multi-sem sync
    # waits into forms walrus codegen accepts.
    if cp is None:
        cp = c
    nc = bacc.Bacc(None, target_bir_lowering=False)
    logits = nc.dram_tensor("logits", [nl, cp], mybir.dt.float32, kind="ExternalInput")
    flatidx = nc.dram_tensor("flatidx", [nl, 1], mybir.dt.int32, kind="ExternalInput")
    nll = nc.dram_tensor("nll", [nl, 1], mybir.dt.float32, kind="ExternalOutput")

    n_tiles = (nl + P - 1) // P
    chunks = [(s, min(f, c - s)) for s in range(0, c, f)]
    nch = len(chunks)

    # Flat [nl*cp, 1] view of logits for the element gather (offset must be 0).
    logits_flat = bass.AP(tensor=logits, offset=0, ap=[[1, nl * cp], [1, 1]])

    with tile.TileContext(nc) as tc:
        with (
            tc.tile_pool(name="chunks", bufs=chunk_bufs) as chunk_pool,
            tc.tile_pool(name="small", bufs=2 * n_tiles) as small,
            tc.tile_pool(name="pscratch", bufs=psum_bufs, space="PSUM") as pscratch,
        ):
            def epilogue(t, parts, gat):
                r0 = t * P
                rows = min(P, nl - r0)
                ssum = small.tile([P, 1], mybir.dt.float32, tag="ssum")
                nc.vector.reduce_sum(
                    out=ssum[:rows], in_=parts[:rows, :], axis=mybir.AxisListType.X
                )
                logz = small.tile([P, 1], mybir.dt.float32, tag="logz")
                nc.scalar.activation(
                    out=logz[:rows],
                    in_=ssum[:rows],
                    func=mybir.ActivationFunctionType.Ln,
                )
                res = small.tile([P, 1], mybir.dt.float32, tag="res")
                nc.vector.tensor_sub(res[:rows], logz[:rows], gat[:rows])
                # store via gpsimd's queue so it can't head-of-line block the
                # HWDGE load ring on the sync engine
                nc.gpsimd.dma_start(out=nll[r0 : r0 + rows, :], in_=res[:rows])

            for _ in range(reps):
                stash = []
                for t in range(n_tiles):
                    r0 = t * P
                    rows = min(P, nl - r0)

                    gat = small.tile([P, 1], mybir.dt.float32, tag="gat")
                    if gather:
                        idx = small.tile([P, 1], mybir.dt.int32, tag="idx")
                        nc.gpsimd.dma_start(
                            out=idx[:rows], in_=flatidx[r0 : r0 + rows, :]
                        )
                        nc.gpsimd.indirect_dma_start(
                            out=gat[:rows],
                            out_offset=None,
                            in_=logits_flat,
                            in_offset=bass.IndirectOffsetOnAxis(
                                ap=idx[:rows, :1], axis=0
                            ),
                        )
                    else:
                        nc.vector.memset(gat[:rows], 0.0)

                    parts = small.tile([P, nch], mybir.dt.float32, tag="parts")
                    mode = ring_mode or ("alternate" if dual_ring else "single")
                    for k, (s, w) in enumerate(chunks):
                        kk = t * nch + k  # global chunk counter for ring rotation
                        ch = chunk_pool.tile([P, f], mybir.dt.float32, tag="ch")
                        if mode == "split":
                            # both HWDGE rings work the same chunk concurrently
                            h = (w + 1) // 2
                            nc.sync.dma_start(
                                out=ch[:rows, :h], in_=logits[r0 : r0 + rows, s : s + h]
                            )
                            nc.scalar.dma_start(
                                out=ch[:rows, h:w],
                                in_=logits[r0 : r0 + rows, s + h : s + w],
                            )
                        else:
                            if mode == "single":
                                eng = nc.sync
                            elif mode == "alternate":
                                eng = nc.scalar if kk % 2 else nc.sync
                            elif mode == "alt_pool":
                                eng = nc.gpsimd if kk % 2 else nc.sync
                            elif mode == "triple":
                                eng = (nc.sync, nc.scalar, nc.gpsimd)[kk % 3]
                            elif mode == "alt_tile":
                                eng = nc.scalar if t % 2 else nc.sync
                            else:
                                raise ValueError(mode)
                            eng.dma_start(
                                out=ch[:rows, :w], in_=logits[r0 : r0 + rows, s : s + w]
                            )
                        we = w if exp_cols is None else min(exp_cols, w)
                        if psum_out:
                            po = pscratch.tile([P, f], mybir.dt.float32, tag="po")
                            out_ap = po[:rows, :we]
                        else:
                            out_ap = ch[:rows, :we]
                        nc.scalar.activation(
                            out=out_ap,
                            in_=ch[:rows, :we],
                            func=mybir.ActivationFunctionType.Exp,
                            accum_out=parts[:rows, k : k + 1],
                        )
                    if batch_epilogue:
                        stash.append((t, parts, gat))
                    else:
                        epilogue(t, parts, gat)
                for t, parts, gat in stash:
                    epilogue(t, parts, gat)
    nc.finalize()
    return nc


def build_program_flat(
    nl=NL,
    c=C,
    x=12565,  # elements per chunk (per partition); row = g chunks of x
    g=4,  # chunks per (padded) row; must divide P
    chunk_bufs=3,
    reps=1,
    ring_mode="single",  # 'single' | 'alt_tile'
    exp_cols=None,  # timing variant: exp only this many cols per tile
    psum_out=None,  # int: split exp into sub-activations of this width, out -> PSUM
    psum_bufs=1,
):
    """Flat-sequential variant: the core's whole shard is streamed as one
    contiguous byte stream. Rows are padded host-side to cpad = g*x elements
    (pad value 0 -> exp adds g*x-c ~ 3 to a ~83k sum, relative error ~4e-5).
    The [nl, cpad] shard is viewed as [nl*g, x] chunk-major; each [128, x]
    tile is one fully-contiguous DRAM read (consecutive descriptors adjacent),
    so the per-core HBM access pattern is a single sequential scan.

    Row n = chunks g*n .. g*n+g-1 = g adjacent partitions of tile j=n//(P//g).
    Per-partition exp-sums land in parts[128, nt]; a small cross-partition
    regroup (strided SBUF DMAs) + DVE adds produce per-row sums.
    """
    assert P % g == 0
    cpad = g * x
    assert cpad >= c
    nq = nl * g  # total chunks
    assert nq % P == 0
    nt = nq // P  # streaming tiles
    R = P // g  # rows per tile
    nrt = nl // P  # row-tiles for gather/output layout
    assert nt == g * nrt

    nc = bacc.Bacc(None, target_bir_lowering=False)
    logits = nc.dram_tensor("logits", [nq, x], mybir.dt.float32, kind="ExternalInput")
    flatidx = nc.dram_tensor("flatidx", [nl, 1], mybir.dt.int32, kind="ExternalInput")
    nll = nc.dram_tensor("nll", [nl, 1], mybir.dt.float32, kind="ExternalOutput")

    logits_flat = bass.AP(tensor=logits, offset=0, ap=[[1, nq * x], [1, 1]])

    nsub = 1 if psum_out is None else -(-x // psum_out)
    subs = (
        [(0, x)]
        if psum_out is None
        else [(m * psum_out, min(psum_out, x - m * psum_out)) for m in range(nsub)]
    )

    with tile.TileContext(nc) as tc:
        with (
            tc.tile_pool(name="chunks", bufs=chunk_bufs) as chunk_pool,
            tc.tile_pool(name="small", bufs=2) as small,
            tc.tile_pool(name="pscr", bufs=psum_bufs, space="PSUM") as pscr,
        ):
            for _ in range(reps):
                # target-logit gather, in row-tile (p, t) layout
                idx = small.tile([P, nrt], mybir.dt.int32, tag="idx")
                gat = small.tile([P, nrt], mybir.dt.float32, tag="gat")
                for t in range(nrt):
                    nc.gpsimd.dma_start(
                        out=idx[:, t : t + 1], in_=flatidx[t * P : (t + 1) * P, :]
                    )
                    nc.gpsimd.indirect_dma_start(
                        out=gat[:, t : t + 1],
                        out_offset=None,
                        in_=logits_flat,
                        in_offset=bass.IndirectOffsetOnAxis(ap=idx[:, t : t + 1], axis=0),
                    )

                ncols = nt * nsub
                parts = small.tile([P, ncols], mybir.dt.float32, tag="parts")
                for j in range(nt):
                    ch = chunk_pool.tile([P, x], mybir.dt.float32, tag="ch")
                    eng = nc.scalar if (ring_mode == "alt_tile" and j % 2) else nc.sync
                    eng.dma_start(out=ch, in_=logits[j * P : (j + 1) * P, :])
                    if psum_out is None:
                        we = x if exp_cols is None else min(exp_cols, x)
                        nc.scalar.activation(
                            out=ch[:, :we],
                            in_=ch[:, :we],
                            func=mybir.ActivationFunctionType.Exp,
                            accum_out=parts[:, j : j + 1],
                        )
                    else:
                        for mi, (s0, w) in enumerate(subs):
                            po = pscr.tile([P, psum_out], mybir.dt.float32, tag="po")
                            nc.scalar.activation(
                                out=po[:, :w],
                                in_=ch[:, s0 : s0 + w],
                                func=mybir.ActivationFunctionType.Exp,
                                accum_out=parts[:, j * nsub + mi : j * nsub + mi + 1],
                            )

                # regroup: re4[a, i*ncols + q] = parts[g*a + i, q]
                re4 = small.tile([R, g * ncols], mybir.dt.float32, tag="re4")
                for i in range(g):
                    nc.gpsimd.dma_start(
                        out=re4[:, i * ncols : (i + 1) * ncols], in_=parts[i::g, :]
                    )
                Ssub = small.tile([R, ncols], mybir.dt.float32, tag="Ssub")
                nc.vector.tensor_add(Ssub, re4[:, 0:ncols], re4[:, ncols : 2 * ncols])
                for i in range(2, g):
                    nc.vector.tensor_add(
                        Ssub, Ssub, re4[:, i * ncols : (i + 1) * ncols]
                    )
                if nsub == 1:
                    S = Ssub
                else:
                    # S[a, j] = sum_m Ssub[a, j*nsub + m]
                    S = small.tile([R, nt], mybir.dt.float32, tag="S")
                    nc.vector.tensor_add(S, Ssub[:, 0::nsub], Ssub[:, 1::nsub])
                    for m in range(2, nsub):
                        nc.vector.tensor_add(S, S, Ssub[:, m::nsub])
                lnS = small.tile([R, nt], mybir.dt.float32, tag="lnS")
                nc.scalar.activation(
                    out=lnS, in_=S, func=mybir.ActivationFunctionType.Ln
                )
                # rearrange (a, j) -> row-tile (p, t): lnS_pt[R*i + a, t] = lnS[a, g*t + i]
                lnS_pt = small.tile([P, nrt], mybir.dt.float32, tag="lnpt")
                for i in range(g):
                    nc.gpsimd.dma_start(
                        out=lnS_pt[i * R : (i + 1) * R, :], in_=lnS[:, i::g]
                    )
                res = small.tile([P, nrt], mybir.dt.float32, tag="res")
                nc.vector.tensor_sub(res, lnS_pt, gat)
                # nll[P*t + p] = res[p, t]
                nll_view = bass.AP(tensor=nll, offset=0, ap=[[1, P], [P, nrt]])
                nc.gpsimd.dma_start(out=nll_view, in_=res)
    nc.finalize()
    return nc


def _make_in_maps_flat(logits, target, x=12565, g=4):
    cpad = g * x
    logits = np.ascontiguousarray(logits, dtype=np.float32)
    padded = np.zeros((N, cpad), dtype=np.float32)
    padded[:, :C] = logits
    tgt = np.asarray(target).astype(np.int64).reshape(N)
    base = np.arange(NL, dtype=np.int64) * cpad
    in_maps = []
    for cid in range(NCORES):
        lo = cid * NL
        lg = padded[lo : lo + NL].reshape(NL * g, x)
        fi = (base + tgt[lo : lo + NL]).astype(np.int32).reshape(NL, 1)
        in_maps.append({"logits": lg, "flatidx": fi})
    return in_maps


_PROG = None

# Winning configuration used by kernel() and test.py's timing runs.
# kind: 'legacy' -> build_program/_make_in_maps, 'flat' -> build_program_flat/_make_in_maps_flat
# flat x=3142/g=16/bufs=12: ~305-315us/rep vs ~355us for the legacy row-chunk
# layout (28-sweep interleaved medians; see NOTES.md).
BEST = {"kind": "flat", "cfg": {"x": 3142, "g": 16, "chunk_bufs": 12}}


def build_timing_program(reps=1):
    build = build_program_flat if BEST["kind"] == "flat" else build_program
    return build(reps=reps, **BEST["cfg"])


def make_in_maps(logits, target):
    if BEST["kind"] == "flat":
        cfg = BEST["cfg"]
        return _make_in_maps_flat(
            logits, target, x=cfg.get("x", 12565), g=cfg.get("g", 4)
        )
    return _make_in_maps(logits, target, cp=BEST["cfg"].get("cp"))


def _get_prog():
    global _PROG
    if _PROG is None:
        _PROG = build_timing_program()
    return _PROG


def _make_in_maps(logits, target, cp=None):
    logits = np.ascontiguousarray(logits, dtype=np.float32)
    if cp is not None and cp != C:
        padded = np.zeros((N, cp), dtype=np.float32)
        padded[:, :C] = logits
        logits = padded
    stride = cp if cp is not None else C
    tgt = np.asarray(target).astype(np.int64).reshape(N)
    base = np.arange(NL, dtype=np.int64) * stride
    in_maps = []
    for cid in range(NCORES):
        lo = cid * NL
        fi = (base + tgt[lo : lo + NL]).astype(np.int32).reshape(NL, 1)
        in_maps.append({"logits": logits[lo : lo + NL], "flatidx": fi})
    return in_maps


def run(logits, target, trace=False):
    """Run on 8 cores; returns (nll [N] f32, BassKernelResults)."""
    nc = _get_prog()
    in_maps = make_in_maps(logits, target)
    br = run_bass_kernel_spmd(nc, in_maps, list(range(NCORES)), trace=trace)
    out = np.concatenate([r["nll"].reshape(NL) for r in br.results], axis=0)
    return out.astype(np.float32, copy=False), br


def kernel(logits, target):
    out, _ = run(logits, target)
    return out

